# revision 1
# baseline (speedup 1.0000x reference)
"""Trainium2 Bass kernel for nn_ExactAttention (block-diagonal sparse attention).

Reference computes dense softmax attention over [N,N] then masks to
block-diagonal segments (batch_seg is sorted).  Only the diagonal blocks
survive, so we compute segment-local attention only.

The reference subtracts the *global* max of Q@K^T before exp; softmax is
shift-invariant except through EPS=1e-8, whose effect is ~1e-8 relative
(denominators are O(100+)), far below fp32 noise, so we skip the max
entirely (max |dot| ~ 70 -> exp(70/sqrt(128)) ~ 450, no overflow).

Sharding: segments are sorted by length (desc) and dealt round-robin:
slot j of every core gets one of ranks [8j, 8j+8), all padded to the
group max L_j, so all 8 cores run one SPMD program with near-zero
padding waste and balanced work.

Precision/perf choices:
  * scores via bf16 hi/lo splitting (host-side):  K.Q^T = Kh.Qh + Kh.Ql
    + Kl.Qh, three full-rate (1 cycle/row) bf16 matmuls accumulated in
    fp32 PSUM.  Error ~2^-17 relative on the dot product (the dropped
    Kl.Ql term), below fp32 matmul noise.  4/3x faster than the PE's
    native fp32 mode (two half-rate passes).
  * AV in native fp32, V-stationary (one weight per key chunk):
    O^T [128 x m] += V_c^T P_c — keeps weight loads minimal and output
    in a layout the host can cheaply transpose.
  * denominator: S = sum_c P_c on DVE; host sums the 128 partitions.
    Padded key rows (zero K) give exp(0)=1; the host subtracts (L-len).
  * PE HAM warm-up: junk bf16 matmuls bridge the DMA wait so the clock
    throttle releases before real matmuls arrive.
"""

import numpy as np
import ml_dtypes

import concourse.bass as bass
import concourse.mybir as mybir
import concourse.tile as tile
from concourse import bacc
from concourse import bass_utils

D = 128
N_CORES = 8
EPS = 1e-8
F32 = mybir.dt.float32
BF16 = mybir.dt.bfloat16
BF16_NP = ml_dtypes.bfloat16

_program_cache = {}


def _build_program(slot_lens):
    """Build + compile the SPMD program for per-slot padded lengths."""
    key = tuple(slot_lens)
    if key in _program_cache:
        return _program_cache[key]

    scale = float(1.0 / np.sqrt(np.float32(D)))
    R = sum(slot_lens)
    offs = np.concatenate([[0], np.cumsum(slot_lens)]).astype(int)
    nkcs = [(L + 127) // 128 for L in slot_lens]
    choffs = np.concatenate([[0], np.cumsum(nkcs)]).astype(int)
    C = int(choffs[-1])
    max_nkc = max(nkcs)

    nc = bacc.Bacc("TRN2", target_bir_lowering=False, debug=False,
                   num_devices=N_CORES)

    # packed [qh | ql | kh | kl] per slot: big contiguous per-partition runs
    qk_d = nc.dram_tensor("qk", [D, 4 * R], BF16, kind="ExternalInput").ap()
    vx_d = nc.dram_tensor("vx", [D, C * 128], F32, kind="ExternalInput").ap()
    # merged [O^T | S] output: slot j, qblock qb0 at cols 2*offs[j]+2*qb0
    os_d = nc.dram_tensor("os", [D, 2 * R], F32, kind="ExternalOutput").ap()

    with tile.TileContext(nc) as tc:
        with tc.tile_pool(name="qk", bufs=2) as qk_pool, \
             tc.tile_pool(name="v", bufs=2) as v_pool, \
             tc.tile_pool(name="p", bufs=2 * max_nkc + 2) as p_pool, \
             tc.tile_pool(name="osb", bufs=3) as o_pool, \
             tc.tile_pool(name="tps", bufs=4, space="PSUM") as t_psum, \
             tc.tile_pool(name="ops", bufs=3, space="PSUM") as o_psum:

            # PE warm-up: HAM releases the clock throttle only after ~3.4us
            # of sustained PE activity; junk bf16 matmuls bridge the initial
            # DMA wait so real matmuls start at 2.4GHz.
            with tc.tile_pool(name="warm", bufs=1) as warm_pool, \
                 tc.tile_pool(name="warmps", bufs=1, space="PSUM") as warm_psum:
                wsb = warm_pool.tile([128, 128], BF16)
                nc.vector.memset(wsb[:], 0.0)
                wps = warm_psum.tile([128, 128], F32)
                for _ in range(24):
                    nc.tensor.matmul(wps[:], wsb[:, :128], wsb[:],
                                     start=True, stop=True)

            for s, L in enumerate(slot_lens):
                nkc = nkcs[s]
                o0 = int(offs[s])
                c0 = int(choffs[s])
                qk_sb = qk_pool.tile([D, 4 * L], BF16, tag="qk")
                vs = v_pool.tile([D, nkc * 128], F32, tag="v")
                # big-burst DMA for all four Q/K pieces, split across both
                # HWDGE queues (sync+scalar) for 2x queue bandwidth; V on the
                # gpsimd SWDGE queue for extra DMA-queue parallelism.
                # Pack order is [qh | kh | ql | kl]; the first sync piece
                # carries exactly what the first score matmul needs (qh +
                # kh chunk0) so the PE can start as early as possible.
                nc.sync.dma_start(qk_sb[:, :2 * L], qk_d[:, 4 * o0:4 * o0 + 2 * L])
                nc.scalar.dma_start(qk_sb[:, 2 * L:],
                                    qk_d[:, 4 * o0 + 2 * L:4 * (o0 + L)])
                # slot 0's V gates the first AV group; SWDGE's ~1us+ first
                # byte would stall the PE there, so keep it on HWDGE
                v_src = vx_d[:, c0 * 128:(c0 + nkc) * 128]
                if s == 0:
                    nc.scalar.dma_start(vs[:], v_src)
                else:
                    nc.gpsimd.dma_start(vs[:], v_src)
                qhs = qk_sb[:, 0:L]
                khs = qk_sb[:, L:2 * L]
                qls = qk_sb[:, 2 * L:3 * L]
                kls = qk_sb[:, 3 * L:4 * L]

                # query blocks of <=512 (PSUM bank limit / moving-max)
                for qb0 in range(0, L, 512):
                    qbs = min(512, L - qb0)
                    p_tiles = []
                    for c in range(nkc):
                        ck = min(128, L - c * 128)
                        kslc = slice(c * 128, c * 128 + ck)
                        qslc = slice(qb0, qb0 + qbs)
                        t_ps = t_psum.tile([128, qbs], F32, tag="t")
                        nc.tensor.matmul(t_ps[:ck, :], khs[:, kslc],
                                         qhs[:, qslc], start=True, stop=False)
                        nc.tensor.matmul(t_ps[:ck, :], khs[:, kslc],
                                         qls[:, qslc], start=False, stop=False)
                        nc.tensor.matmul(t_ps[:ck, :], kls[:, kslc],
                                         qhs[:, qslc], start=False, stop=True)
                        p_sb = p_pool.tile([128, qbs], F32, tag="p")
                        nc.scalar.activation(p_sb[:ck, :], t_ps[:ck, :],
                                             mybir.ActivationFunctionType.Exp,
                                             scale=scale)
                        p_tiles.append(p_sb)

                    # AV: O^T += V_c^T P_c  (V stationary, one weight/chunk)
                    o_ps = o_psum.tile([128, qbs], F32, tag="ops")
                    for c in range(nkc):
                        ck = min(128, L - c * 128)
                        nc.tensor.matmul(o_ps[:],
                                         vs[:ck, c * 128:(c + 1) * 128],
                                         p_tiles[c][:ck, :],
                                         start=(c == 0), stop=(c == nkc - 1))

                    # Merged [O^T | S] tile.  S = sum_c P_c (DVE); host sums
                    # partitions for den.  Only the valid [:ck] partitions of
                    # each P tile are written by exp; partial chunks are
                    # slice-added so stale partitions never leak into S.
                    os_sb = o_pool.tile([128, 2 * qbs], F32, tag="o")
                    s_ap = os_sb[:, qbs:2 * qbs]
                    sck0 = min(128, L)
                    if sck0 < 128:
                        nc.gpsimd.memset(s_ap, 0.0)
                        nc.vector.tensor_add(s_ap[:sck0, :], s_ap[:sck0, :],
                                             p_tiles[0][:sck0, :])
                    else:
                        nc.vector.tensor_copy(s_ap, p_tiles[0][:])
                    for c in range(1, nkc):
                        ck = min(128, L - c * 128)
                        nc.vector.tensor_add(s_ap[:ck, :], s_ap[:ck, :],
                                             p_tiles[c][:ck, :])
                    d0 = 2 * o0 + 2 * qb0
                    # S can ship as soon as the adds finish (often before AV)
                    nc.sync.dma_start(os_d[:, d0 + qbs:d0 + 2 * qbs], s_ap)
                    # O^T copy+store split across engine pairs (DVE+sync,
                    # ACT+scalar) so the tail chains run in parallel
                    h = qbs // 2
                    nc.vector.tensor_copy(os_sb[:, :h], o_ps[:, :h])
                    nc.sync.dma_start(os_d[:, d0:d0 + h], os_sb[:, :h])
                    nc.scalar.copy(os_sb[:, h:qbs], o_ps[:, h:])
                    nc.scalar.dma_start(os_d[:, d0 + h:d0 + qbs],
                                        os_sb[:, h:qbs])

    nc.compile()
    _program_cache[key] = nc
    return nc


def _reference_host(Q, K, V, num_batch, batch_seg):
    """Pure-NumPy fallback for input shapes outside the tuned envelope."""
    dot = Q.astype(np.float64) @ K.T.astype(np.float64)
    A = np.exp((dot - dot.max()) / np.sqrt(np.float64(Q.shape[-1])))
    if num_batch > 1:
        A = np.where(batch_seg[None, :] == batch_seg[:, None], A, 0.0)
    return ((A / (A.sum(-1, keepdims=True) + EPS)) @ V.astype(np.float64)
            ).astype(np.float32)


def kernel(Q, K, V, num_batch, batch_seg):
    Q = np.asarray(Q, dtype=np.float32)
    K = np.asarray(K, dtype=np.float32)
    V = np.asarray(V, dtype=np.float32)
    batch_seg = np.asarray(batch_seg)
    N = Q.shape[0]
    nb = int(num_batch)

    counts = np.bincount(batch_seg.astype(np.int64), minlength=max(nb, 1))
    if nb < 2 or (counts.max() if nb else N) > 2048:
        return _reference_host(Q, K, V, nb, batch_seg)

    # row indices per segment (robust to unsorted batch_seg)
    row_order = np.argsort(batch_seg, kind="stable")
    starts = np.zeros(nb + 1, dtype=np.int64)
    np.cumsum(counts, out=starts[1:])

    # rank segments by length desc, group into slots of 8, then order slots
    # smallest-first: the first slot runs while the PE clock is still cold
    # and its load gates the pipeline start, so make it the cheapest.
    order = np.argsort(-counts, kind="stable")
    n_slots = (nb + N_CORES - 1) // N_CORES
    groups = []  # ascending by group max length
    for j in range(n_slots):
        grp = order[(n_slots - 1 - j) * N_CORES:(n_slots - j) * N_CORES]
        groups.append((max(1, int(counts[grp].max())), grp))
    # ascending by length: smallest slot first — it runs while the PE clock
    # is still cold and its load gates the pipeline start (measured better
    # than putting the largest slot in the middle or first)
    perm = list(range(n_slots))
    slot_lens = []
    assign = {}  # (core, slot) -> seg id
    for j, gi in enumerate(perm):
        L, grp = groups[gi]
        slot_lens.append(L)
        for c, seg in enumerate(grp):
            assign[(c, j)] = int(seg)

    offs = np.concatenate([[0], np.cumsum(slot_lens)]).astype(int)
    nkcs = [(L + 127) // 128 for L in slot_lens]
    choffs = np.concatenate([[0], np.cumsum(nkcs)]).astype(int)
    R = int(offs[-1])
    C = int(choffs[-1])

    nc = _build_program(tuple(slot_lens))

    in_maps = []
    for core in range(N_CORES):
        Qp = np.zeros((R, D), np.float32)
        Kp = np.zeros((R, D), np.float32)
        Vp = np.zeros((C * 128, D), np.float32)
        for j in range(n_slots):
            seg = assign.get((core, j))
            if seg is None:
                continue
            b0, b1 = starts[seg], starts[seg + 1]
            ln = int(b1 - b0)
            if ln == 0:
                continue
            ridx = row_order[b0:b1]
            o0 = int(offs[j])
            Qp[o0:o0 + ln] = Q[ridx]
            Kp[o0:o0 + ln] = K[ridx]
            v0 = int(choffs[j]) * 128
            Vp[v0:v0 + ln] = V[ridx]
        qt = np.ascontiguousarray(Qp.T)
        kt = np.ascontiguousarray(Kp.T)
        qh = qt.astype(BF16_NP)
        ql = (qt - qh.astype(np.float32)).astype(BF16_NP)
        kh = kt.astype(BF16_NP)
        kl = (kt - kh.astype(np.float32)).astype(BF16_NP)
        qk = np.empty((D, 4 * R), BF16_NP)
        for j in range(n_slots):
            o0, L = int(offs[j]), slot_lens[j]
            qk[:, 4 * o0:4 * o0 + L] = qh[:, o0:o0 + L]
            qk[:, 4 * o0 + L:4 * o0 + 2 * L] = kh[:, o0:o0 + L]
            qk[:, 4 * o0 + 2 * L:4 * o0 + 3 * L] = ql[:, o0:o0 + L]
            qk[:, 4 * o0 + 3 * L:4 * o0 + 4 * L] = kl[:, o0:o0 + L]
        vh = np.ascontiguousarray(
            Vp.reshape(C, 128, D).transpose(1, 0, 2)).reshape(D, C * 128)
        in_maps.append({
            "qk": qk, "vx": vh,
        })

    global _last_in_maps
    _last_in_maps = in_maps
    res = bass_utils.run_bass_kernel_spmd(nc, in_maps,
                                          core_ids=list(range(N_CORES)))

    out = np.empty((N, D), np.float32)
    for (core, j), seg in assign.items():
        b0, b1 = starts[seg], starts[seg + 1]
        ln = int(b1 - b0)
        if ln == 0:
            continue
        o0 = int(offs[j])
        L = slot_lens[j]
        osr = res.results[core]["os"]                       # [D, 2R]
        # unpack per-qblock [ot(qbs) | s(qbs)] layout
        otT = np.empty((D, L), np.float32)
        sS = np.empty((D, L), np.float32)
        for qb0 in range(0, L, 512):
            qbs = min(512, L - qb0)
            d0 = 2 * o0 + 2 * qb0
            otT[:, qb0:qb0 + qbs] = osr[:, d0:d0 + qbs]
            sS[:, qb0:qb0 + qbs] = osr[:, d0 + qbs:d0 + 2 * qbs]
        # padded keys contribute exp(0)=1 each to the raw column sums
        den = sS[:, :ln].sum(axis=0, dtype=np.float64) - float(L - ln) + EPS
        out[row_order[b0:b1]] = (otT[:, :ln].T / den[:, None]).astype(np.float32)
    return out



# revision 3
# speedup vs baseline: 1.2990x; 1.2990x over previous
"""Trainium2 Bass kernel for nn_ExactAttention (block-diagonal sparse attention).

Reference computes dense softmax attention over [N,N] then masks to
block-diagonal segments (batch_seg is sorted).  Only the diagonal blocks
survive, so we compute segment-local attention only.

The reference subtracts the *global* max of Q@K^T before exp; softmax is
shift-invariant except through EPS=1e-8, whose effect is ~1e-8 relative
(denominators are O(100+)), far below the 2e-2 gate, so we skip the max
entirely (max |dot| ~ 70 -> exp(70/sqrt(128)) ~ 450, no overflow).

Sharding: segments are sorted by length (desc) and dealt round-robin:
slot j of every core gets one of ranks [8j, 8j+8), all padded to the
group max L_j, so all 8 cores run one SPMD program with near-zero
padding waste and balanced work.

Precision/perf (vs the fp32/hi-lo predecessor, 37.1us -> target ~2x):
  * rel-err budget is 2e-2; an all-fp16 pipeline measures 5.5e-4 on the
    real inputs, so every matmul runs at the PE's full 1 col/cycle rate:
      - scores: ONE fp16 matmul (was 3 bf16 hi/lo passes)
      - AV:     fp16 (was native fp32 = 4 col-cycles each)
  * softmax denominator on the PE: den[1,L] += ones[ck,1]^T @ P_c
    accumulated across key chunks.  Replaces the DVE add-chain, the
    [128,R] partial-sum DMA, and the host partition-reduction.
  * zero-padded K rows give exp(0)=1 per padded key; the host subtracts
    (L - len) from den exactly.  Zero-padded V rows keep AV clean.
  * software pipelining: slot j+1's score matmuls are emitted BEFORE
    slot j's AV/den matmuls so the PE works while ACT runs exp(j).
  * outputs in fp16 (O^T) + fp32 (den); host divides and scatters.
  * PE HAM warm-up: junk bf16 matmuls bridge the first DMA wait so the
    clock throttle releases before real matmuls arrive.
"""

import numpy as np

import concourse.bass as bass
import concourse.mybir as mybir
import concourse.tile as tile
from concourse import bacc
from concourse import bass_utils

D = 128
N_CORES = 8
EPS = 1e-8
F32 = mybir.dt.float32
F16 = mybir.dt.float16
BF16 = mybir.dt.bfloat16

_program_cache = {}
_last_in_maps = None


def _build_program(slot_lens):
    """Build + compile the SPMD program for per-slot padded lengths."""
    key = tuple(slot_lens)
    if key in _program_cache:
        return _program_cache[key]

    scale = float(1.0 / np.sqrt(np.float32(D)))
    R = sum(slot_lens)
    offs = np.concatenate([[0], np.cumsum(slot_lens)]).astype(int)
    nkcs = [(L + 127) // 128 for L in slot_lens]
    choffs = np.concatenate([[0], np.cumsum(nkcs)]).astype(int)
    C = int(choffs[-1])

    nc = bacc.Bacc("TRN2", target_bir_lowering=False, debug=False,
                   num_devices=N_CORES)

    # packed [q | k] per slot (fp16): slot j at cols 2*offs[j]
    qk_d = nc.dram_tensor("qk", [D, 2 * R], F16, kind="ExternalInput").ap()
    # V chunk-major: [key-in-chunk, chunk*128 + d]
    vx_d = nc.dram_tensor("vx", [128, C * 128], F16, kind="ExternalInput").ap()
    o_d = nc.dram_tensor("o", [D, R], F16, kind="ExternalOutput").ap()
    den_d = nc.dram_tensor("den", [1, R], F32, kind="ExternalOutput").ap()

    with tile.TileContext(nc) as tc:
        with tc.tile_pool(name="qk", bufs=1) as qk_pool, \
             tc.tile_pool(name="v", bufs=1) as v_pool, \
             tc.tile_pool(name="p", bufs=9) as p_pool, \
             tc.tile_pool(name="ob", bufs=2) as o_pool, \
             tc.tile_pool(name="dn", bufs=1) as dn_pool, \
             tc.tile_pool(name="one", bufs=1) as one_pool, \
             tc.tile_pool(name="tps", bufs=4, space="PSUM") as t_psum, \
             tc.tile_pool(name="ops", bufs=2, space="PSUM") as o_psum, \
             tc.tile_pool(name="dps", bufs=1, space="PSUM") as d_psum:

            ones_sb = one_pool.tile([128, 1], F16)
            nc.vector.memset(ones_sb[:], 1.0)
            den_sb = dn_pool.tile([1, R], F32)

            # PE warm-up: HAM releases the clock throttle only after ~3.4us
            # of sustained PE activity; junk bf16 matmuls bridge the initial
            # DMA wait so real matmuls run at 2.4GHz as soon as data lands.
            with tc.tile_pool(name="warm", bufs=1) as warm_pool, \
                 tc.tile_pool(name="warmps", bufs=1, space="PSUM") as warm_psum:
                wsb = warm_pool.tile([128, 128], BF16)
                nc.gpsimd.memset(wsb[:], 0.0)
                wps = warm_psum.tile([128, 128], F32)
                for _ in range(12):
                    nc.tensor.matmul(wps[:], wsb[:, :128], wsb[:],
                                     start=True, stop=True)

            # input DMAs: per-slot qk on sync (precise deps, first slot
            # lands first), whole V in one burst on scalar (needed only by
            # the first AV group, ~4us in).
            qk_tiles = []
            for j, L in enumerate(slot_lens):
                t = qk_pool.tile([D, 2 * L], F16, tag=f"qk{j}")
                nc.sync.dma_start(t[:], qk_d[:, 2 * offs[j]:2 * (offs[j] + L)])
                qk_tiles.append(t)
            v_sb = v_pool.tile([128, C * 128], F16)
            nc.scalar.dma_start(v_sb[:], vx_d[:])

            def emit_av_s(j, L, nkc, o0, c0, p_tiles):
                o_ps = o_psum.tile([128, 512], F32, tag="o")
                for (c, ck, p_sb) in p_tiles:
                    nc.tensor.matmul(o_ps[:, :L],
                                     v_sb[:ck, (c0 + c) * 128:(c0 + c + 1) * 128],
                                     p_sb[:ck, :L],
                                     start=(c == 0), stop=(c == nkc - 1))
                d_ps = d_psum.tile([128, 512], F32, tag="d")
                for (c, ck, p_sb) in p_tiles:
                    nc.tensor.matmul(d_ps[:1, :L], ones_sb[:ck, :],
                                     p_sb[:ck, :L],
                                     start=(c == 0), stop=(c == nkc - 1))
                o_sb = o_pool.tile([128, 512], F16, tag="ob")
                nc.vector.tensor_copy(o_sb[:, :L], o_ps[:, :L])
                nc.sync.dma_start(o_d[:, o0:o0 + L], o_sb[:, :L])
                nc.vector.tensor_copy(den_sb[:, o0:o0 + L], d_ps[:1, :L])

            pending = None
            for j, L in enumerate(slot_lens):
                nkc = nkcs[j]
                o0 = int(offs[j])
                c0 = int(choffs[j])
                q_ap = qk_tiles[j][:, 0:L]
                k_ap = qk_tiles[j][:, L:2 * L]
                p_tiles = []
                for c in range(nkc):
                    ck = min(128, L - c * 128)
                    t_ps = t_psum.tile([128, 512], F32, tag="t")
                    nc.tensor.matmul(t_ps[:ck, :L],
                                     k_ap[:, c * 128:c * 128 + ck], q_ap,
                                     start=True, stop=True)
                    p_sb = p_pool.tile([128, 512], F16, tag="p")
                    nc.scalar.activation(p_sb[:ck, :L], t_ps[:ck, :L],
                                         mybir.ActivationFunctionType.Exp,
                                         scale=scale)
                    p_tiles.append((c, ck, p_sb))
                if pending is not None:
                    emit_av_s(*pending)
                pending = (j, L, nkc, o0, c0, p_tiles)
            emit_av_s(*pending)
            nc.scalar.dma_start(den_d[:], den_sb[:])

    nc.compile()
    _program_cache[key] = nc
    return nc


def _reference_host(Q, K, V, num_batch, batch_seg):
    """Pure-NumPy fallback for input shapes outside the tuned envelope."""
    dot = Q.astype(np.float64) @ K.T.astype(np.float64)
    A = np.exp((dot - dot.max()) / np.sqrt(np.float64(Q.shape[-1])))
    if num_batch > 1:
        A = np.where(batch_seg[None, :] == batch_seg[:, None], A, 0.0)
    return ((A / (A.sum(-1, keepdims=True) + EPS)) @ V.astype(np.float64)
            ).astype(np.float32)


def kernel(Q, K, V, num_batch, batch_seg):
    Q = np.asarray(Q, dtype=np.float32)
    K = np.asarray(K, dtype=np.float32)
    V = np.asarray(V, dtype=np.float32)
    batch_seg = np.asarray(batch_seg)
    N = Q.shape[0]
    nb = int(num_batch)

    counts = np.bincount(batch_seg.astype(np.int64), minlength=max(nb, 1))
    if nb < 2 or (counts.max() if nb else N) > 512:
        return _reference_host(Q, K, V, nb, batch_seg)

    # row indices per segment (robust to unsorted batch_seg)
    row_order = np.argsort(batch_seg, kind="stable")
    starts = np.zeros(nb + 1, dtype=np.int64)
    np.cumsum(counts, out=starts[1:])

    # rank segments by length desc, group into slots of 8, order slots
    # ascending by group max: the first slot runs while the PE clock is
    # still cold and its load gates the pipeline start.
    order = np.argsort(-counts, kind="stable")
    n_slots = (nb + N_CORES - 1) // N_CORES
    slot_lens = []
    assign = {}  # (core, slot) -> seg id
    for j in range(n_slots):
        grp = order[(n_slots - 1 - j) * N_CORES:(n_slots - j) * N_CORES]
        slot_lens.append(max(1, int(counts[grp].max())))
        for c, seg in enumerate(grp):
            assign[(c, j)] = int(seg)

    offs = np.concatenate([[0], np.cumsum(slot_lens)]).astype(int)
    nkcs = [(L + 127) // 128 for L in slot_lens]
    choffs = np.concatenate([[0], np.cumsum(nkcs)]).astype(int)
    R = int(offs[-1])
    C = int(choffs[-1])

    nc = _build_program(tuple(slot_lens))

    in_maps = []
    for core in range(N_CORES):
        Qp = np.zeros((R, D), np.float32)
        Kp = np.zeros((R, D), np.float32)
        Vp = np.zeros((C * 128, D), np.float32)
        for j in range(n_slots):
            seg = assign.get((core, j))
            if seg is None:
                continue
            b0, b1 = starts[seg], starts[seg + 1]
            ln = int(b1 - b0)
            if ln == 0:
                continue
            ridx = row_order[b0:b1]
            o0 = int(offs[j])
            Qp[o0:o0 + ln] = Q[ridx]
            Kp[o0:o0 + ln] = K[ridx]
            v0 = int(choffs[j]) * 128
            Vp[v0:v0 + ln] = V[ridx]
        qt = Qp.T.astype(np.float16)
        kt = Kp.T.astype(np.float16)
        qk = np.empty((D, 2 * R), np.float16)
        for j in range(n_slots):
            o0, L = int(offs[j]), slot_lens[j]
            qk[:, 2 * o0:2 * o0 + L] = qt[:, o0:o0 + L]
            qk[:, 2 * o0 + L:2 * (o0 + L)] = kt[:, o0:o0 + L]
        vh = np.ascontiguousarray(
            Vp.reshape(C, 128, D).transpose(1, 0, 2)
        ).reshape(128, C * D).astype(np.float16)
        in_maps.append({"qk": qk, "vx": vh})

    global _last_in_maps
    _last_in_maps = in_maps
    res = bass_utils.run_bass_kernel_spmd(nc, in_maps,
                                          core_ids=list(range(N_CORES)))

    out = np.empty((N, D), np.float32)
    for (core, j), seg in assign.items():
        b0, b1 = starts[seg], starts[seg + 1]
        ln = int(b1 - b0)
        if ln == 0:
            continue
        o0 = int(offs[j])
        L = slot_lens[j]
        otT = res.results[core]["o"][:, o0:o0 + ln].astype(np.float32)
        den_raw = res.results[core]["den"][0, o0:o0 + ln].astype(np.float64)
        # padded keys (zero K) contribute exp(0)=1 each to the device den
        den = den_raw - float(L - ln) + EPS
        out[row_order[b0:b1]] = (otT / den[None, :]).T.astype(np.float32)
    return out


# revision 4
# speedup vs baseline: 1.3271x; 1.0216x over previous
"""Trainium2 Bass kernel for nn_ExactAttention (block-diagonal sparse attention).

Reference computes dense softmax attention over [N,N] then masks to
block-diagonal segments (batch_seg is sorted).  Only the diagonal blocks
survive, so we compute segment-local attention only.

The reference subtracts the *global* max of Q@K^T before exp; softmax is
shift-invariant except through EPS=1e-8, whose effect is ~1e-8 relative
(denominators are O(100+)), far below the 2e-2 gate, so we skip the max
entirely (max |dot| ~ 70 -> exp(70/sqrt(128)) ~ 450, no overflow).

Sharding: segments are sorted by length (desc) and dealt round-robin:
slot j of every core gets one of ranks [8j, 8j+8), all padded to the
group max L_j, so all 8 cores run one SPMD program with near-zero
padding waste and balanced work.

Perf design (measured 37.1us baseline -> this):
  * rel-err budget is 2e-2; an all-fp16 pipeline measures 5.5e-4 on the
    real inputs, so every matmul runs at the PE's full 1 col/cycle rate:
    scores in ONE fp16 matmul (was 3 bf16 hi/lo passes), AV in fp16
    (was native fp32 = 4 col-cycles/col).
  * softmax denominator on the PE: den[1,L] += ones[ck,1]^T @ P_c.
    Zero-padded K rows give exp(0)=1 per padded key; the host subtracts
    (L - len) exactly.  Zero-padded V rows keep AV clean.
  * ACTIVATE costs (N+352)/1.2 ns and does NOT pipeline its fixed part,
    so exps are pair-merged: chunks land 512-aligned in a 2-bank PSUM
    tile and one EXP covers both (garbage cols are never read).
  * software pipelining: slot j+1's score matmuls are emitted BEFORE
    slot j's AV/den matmuls so the PE works while ACT runs exp(j).
  * q-halves on the sync HWDGE queue, k-halves + V on the scalar queue:
    two queues halve the first-data latency; outputs (den then O^T, per
    slot) go on sync.  No SWDGE use (its drains lengthen the epilogue).
  * slot order [2nd-largest, ..., largest, smallest]: the early slots
    eat the cold-clock window (HAM un-throttles after ~3.4us of PE
    activity; ~26 junk matmuls bridge the initial DMA wait), and the
    smallest slot's AV->cast->DMA chain ends the kernel.
  * outputs in fp16 (O^T) + fp32 (den); host divides and scatters.
"""

import numpy as np

import concourse.bass as bass
import concourse.mybir as mybir
import concourse.tile as tile
from concourse import bacc
from concourse import bass_utils

D = 128
N_CORES = 8
EPS = 1e-8
F32 = mybir.dt.float32
F16 = mybir.dt.float16
BF16 = mybir.dt.bfloat16

_program_cache = {}
_last_in_maps = None


def _build_program(slot_lens):
    """Build + compile the SPMD program for per-slot padded lengths."""
    key = tuple(slot_lens)
    if key in _program_cache:
        return _program_cache[key]

    scale = float(1.0 / np.sqrt(np.float32(D)))
    R = sum(slot_lens)
    offs = np.concatenate([[0], np.cumsum(slot_lens)]).astype(int)
    nkcs = [(L + 127) // 128 for L in slot_lens]
    choffs = np.concatenate([[0], np.cumsum(nkcs)]).astype(int)
    C = int(choffs[-1])

    nc = bacc.Bacc("TRN2", target_bir_lowering=False, debug=False,
                   num_devices=N_CORES)

    # q and k packed separately (fp16): slot j at cols offs[j]
    q_d = nc.dram_tensor("q", [D, R], F16, kind="ExternalInput").ap()
    k_d = nc.dram_tensor("k", [D, R], F16, kind="ExternalInput").ap()
    # V chunk-major: [key-in-chunk, chunk*128 + d]
    vx_d = nc.dram_tensor("vx", [128, C * 128], F16, kind="ExternalInput").ap()
    o_d = nc.dram_tensor("o", [D, R], F16, kind="ExternalOutput").ap()
    den_d = nc.dram_tensor("den", [1, R], F32, kind="ExternalOutput").ap()

    with tile.TileContext(nc) as tc:
        with tc.tile_pool(name="qb", bufs=1) as q_pool, \
             tc.tile_pool(name="kb", bufs=1) as k_pool, \
             tc.tile_pool(name="v", bufs=1) as v_pool, \
             tc.tile_pool(name="p", bufs=5) as p_pool, \
             tc.tile_pool(name="ob", bufs=2) as ob_pool, \
             tc.tile_pool(name="dn", bufs=1) as dn_pool, \
             tc.tile_pool(name="one", bufs=1) as one_pool, \
             tc.tile_pool(name="wrm", bufs=1) as warm_pool, \
             tc.tile_pool(name="tps", bufs=2, space="PSUM") as t_psum, \
             tc.tile_pool(name="ops", bufs=2, space="PSUM") as o_psum, \
             tc.tile_pool(name="dps", bufs=1, space="PSUM") as d_psum:

            ones_sb = one_pool.tile([128, 1], F16)
            nc.vector.memset(ones_sb[:], 1.0)
            den_sb = dn_pool.tile([1, R], F32)

            # PE warm-up: HAM releases the clock throttle only after ~3.4us
            # of sustained PE activity; junk bf16 matmuls bridge the initial
            # DMA wait so real matmuls run at 2.4GHz as soon as data lands.
            wsb = warm_pool.tile([128, 128], BF16)
            nc.vector.memset(wsb[:], 0.0)
            wps = o_psum.tile([128, 128], F32, tag="o")
            for _ in range(13):
                nc.tensor.matmul(wps[:], wsb[:, :128], wsb[:],
                                 start=True, stop=True)
            wps2 = o_psum.tile([128, 128], F32, tag="o")
            for _ in range(13):
                nc.tensor.matmul(wps2[:], wsb[:, :128], wsb[:],
                                 start=True, stop=True)

            # input DMAs: q-halves on sync, k-halves + V on scalar, ordered
            # so slot 0's operands land first and V is in time for AV(0).
            q_tiles, k_tiles = [], []
            for j, L in enumerate(slot_lens):
                qt = q_pool.tile([D, L], F16, tag=f"q{j}")
                kt = k_pool.tile([D, L], F16, tag=f"k{j}")
                q_tiles.append(qt)
                k_tiles.append(kt)
            nc.sync.dma_start(q_tiles[0][:], q_d[:, offs[0]:offs[0] + slot_lens[0]])
            nc.scalar.dma_start(k_tiles[0][:], k_d[:, offs[0]:offs[0] + slot_lens[0]])
            v_sb = v_pool.tile([128, C * 128], F16)
            nc.scalar.dma_start(v_sb[:], vx_d[:])
            for j in range(1, len(slot_lens)):
                L = slot_lens[j]
                nc.sync.dma_start(q_tiles[j][:], q_d[:, offs[j]:offs[j] + L])
                nc.scalar.dma_start(k_tiles[j][:], k_d[:, offs[j]:offs[j] + L])

            def emit_av_s(j, L, nkc, o0, c0, p_tiles):
                o_ps = o_psum.tile([128, 512], F32, tag="o")
                for (c, ck, p_sb, pc) in p_tiles:
                    nc.tensor.matmul(o_ps[:, :L],
                                     v_sb[:ck, (c0 + c) * 128:(c0 + c + 1) * 128],
                                     p_sb[:ck, pc:pc + L],
                                     start=(c == 0), stop=(c == nkc - 1))
                d_ps = d_psum.tile([128, 512], F32, tag="d")
                for (c, ck, p_sb, pc) in p_tiles:
                    nc.tensor.matmul(d_ps[:1, :L], ones_sb[:ck, :],
                                     p_sb[:ck, pc:pc + L],
                                     start=(c == 0), stop=(c == nkc - 1))
                # den first: its (1-partition, latency-bound) DMA overlaps
                # the O^T cast + store that follow.
                nc.vector.tensor_copy(den_sb[:, o0:o0 + L], d_ps[:1, :L])
                nc.sync.dma_start(den_d[:, o0:o0 + L], den_sb[:, o0:o0 + L])
                o_sb = ob_pool.tile([128, 512], F16, tag="ob")
                nc.vector.tensor_copy(o_sb[:, :L], o_ps[:, :L])
                nc.sync.dma_start(o_d[:, o0:o0 + L], o_sb[:, :L])

            pending = None
            for j, L in enumerate(slot_lens):
                nkc = nkcs[j]
                o0 = int(offs[j])
                c0 = int(choffs[j])
                q_ap = q_tiles[j]
                k_ap = k_tiles[j]
                p_tiles = []
                # chunk pairs share a 2-bank PSUM tile (512-col aligned)
                # and ONE merged EXP over [*, :512+L] (the gap cols hold
                # garbage that is never read downstream).
                for c0p in range(0, nkc, 2):
                    npair = min(2, nkc - c0p)
                    t_ps = t_psum.tile([128, 1024], F32, tag="t")
                    p_sb = p_pool.tile([128, 1024], F16, tag="p")
                    for m in range(npair):
                        c = c0p + m
                        ck = min(128, L - c * 128)
                        nc.tensor.matmul(t_ps[:ck, m * 512:m * 512 + L],
                                         k_ap[:, c * 128:c * 128 + ck], q_ap[:],
                                         start=True, stop=True)
                        p_tiles.append((c, ck, p_sb, m * 512))
                    w = (npair - 1) * 512 + L
                    nc.scalar.activation(p_sb[:, :w], t_ps[:, :w],
                                         mybir.ActivationFunctionType.Exp,
                                         scale=scale)
                if pending is not None:
                    emit_av_s(*pending)
                pending = (j, L, nkc, o0, c0, p_tiles)
            emit_av_s(*pending)

    nc.compile()
    _program_cache[key] = nc
    return nc


def _reference_host(Q, K, V, num_batch, batch_seg):
    """Pure-NumPy fallback for input shapes outside the tuned envelope."""
    dot = Q.astype(np.float64) @ K.T.astype(np.float64)
    A = np.exp((dot - dot.max()) / np.sqrt(np.float64(Q.shape[-1])))
    if num_batch > 1:
        A = np.where(batch_seg[None, :] == batch_seg[:, None], A, 0.0)
    return ((A / (A.sum(-1, keepdims=True) + EPS)) @ V.astype(np.float64)
            ).astype(np.float32)


def kernel(Q, K, V, num_batch, batch_seg):
    Q = np.asarray(Q, dtype=np.float32)
    K = np.asarray(K, dtype=np.float32)
    V = np.asarray(V, dtype=np.float32)
    batch_seg = np.asarray(batch_seg)
    N = Q.shape[0]
    nb = int(num_batch)

    counts = np.bincount(batch_seg.astype(np.int64), minlength=max(nb, 1))
    if nb < 2 or (counts.max() if nb else N) > 512:
        return _reference_host(Q, K, V, nb, batch_seg)

    # row indices per segment (robust to unsorted batch_seg)
    row_order = np.argsort(batch_seg, kind="stable")
    starts = np.zeros(nb + 1, dtype=np.int64)
    np.cumsum(counts, out=starts[1:])

    # rank segments by length desc, group into slots of 8.  Slot order:
    # ascending by group max, except the largest group is placed 3rd and
    # the smallest LAST — early slots eat the cold-clock window, and the
    # smallest slot's output chain ends the kernel.
    order = np.argsort(-counts, kind="stable")
    n_slots = (nb + N_CORES - 1) // N_CORES
    groups = []  # (Lmax, members) ascending by Lmax
    for j in range(n_slots):
        grp = order[(n_slots - 1 - j) * N_CORES:(n_slots - j) * N_CORES]
        groups.append((max(1, int(counts[grp].max())), grp))
    perm = list(range(n_slots))
    if n_slots >= 2:
        perm = perm[1:] + perm[:1]  # smallest last, 2nd-smallest first
    slot_lens = []
    assign = {}  # (core, slot) -> seg id
    for j, gi in enumerate(perm):
        Lmax, grp = groups[gi]
        slot_lens.append(Lmax)
        for c, seg in enumerate(grp):
            assign[(c, j)] = int(seg)

    offs = np.concatenate([[0], np.cumsum(slot_lens)]).astype(int)
    nkcs = [(L + 127) // 128 for L in slot_lens]
    choffs = np.concatenate([[0], np.cumsum(nkcs)]).astype(int)
    R = int(offs[-1])
    C = int(choffs[-1])

    nc = _build_program(tuple(slot_lens))

    in_maps = []
    for core in range(N_CORES):
        Qp = np.zeros((R, D), np.float32)
        Kp = np.zeros((R, D), np.float32)
        Vp = np.zeros((C * 128, D), np.float32)
        for j in range(n_slots):
            seg = assign.get((core, j))
            if seg is None:
                continue
            b0, b1 = starts[seg], starts[seg + 1]
            ln = int(b1 - b0)
            if ln == 0:
                continue
            ridx = row_order[b0:b1]
            o0 = int(offs[j])
            Qp[o0:o0 + ln] = Q[ridx]
            Kp[o0:o0 + ln] = K[ridx]
            v0 = int(choffs[j]) * 128
            Vp[v0:v0 + ln] = V[ridx]
        vh = np.ascontiguousarray(
            Vp.reshape(C, 128, D).transpose(1, 0, 2)
        ).reshape(128, C * D).astype(np.float16)
        in_maps.append({
            "q": np.ascontiguousarray(Qp.T).astype(np.float16),
            "k": np.ascontiguousarray(Kp.T).astype(np.float16),
            "vx": vh,
        })

    global _last_in_maps
    _last_in_maps = in_maps
    res = bass_utils.run_bass_kernel_spmd(nc, in_maps,
                                          core_ids=list(range(N_CORES)))

    out = np.empty((N, D), np.float32)
    for (core, j), seg in assign.items():
        b0, b1 = starts[seg], starts[seg + 1]
        ln = int(b1 - b0)
        if ln == 0:
            continue
        o0 = int(offs[j])
        L = slot_lens[j]
        otT = res.results[core]["o"][:, o0:o0 + ln].astype(np.float32)
        den_raw = res.results[core]["den"][0, o0:o0 + ln].astype(np.float64)
        # padded keys (zero K) contribute exp(0)=1 each to the device den
        den = den_raw - float(L - ln) + EPS
        out[row_order[b0:b1]] = (otT / den[None, :]).T.astype(np.float32)
    return out
